# revision 2
# baseline (speedup 1.0000x reference)
"""AttentionBlock (GroupNorm -> conv1d k=32 QKV -> 16-head attention over t=4096
-> conv1d k=5 proj -> residual) on 8 Trainium2 NeuronCores.

Sharding: 16 (batch, head) attention tasks over 8 cores -> 2 heads per core
(core c: batch c//2, heads 2*(c%2), 2*(c%2)+1). Each core computes GroupNorm
for its batch, the QKV conv restricted to its heads' 768 output channels
(expressed as 128 accumulated matmuls per PSUM tile), and attention for its 2
heads. The proj conv needs all 512 h-channels, so core pairs AllGather h, then
each core computes a 256-output-channel half of the proj conv + residual.

Matmuls run in float32r (full PE rate at N=512, ~1e-4 relative error);
attention probabilities/AV run in bf16. Host pre-rounds f32r weights
(RNE to 11 mantissa bits, verified bit-exact against the device cast).
"""

import functools
import numpy as np

import concourse.bass as bass
import concourse.mybir as mybir
import concourse.tile as tile
from concourse import bass_utils
from concourse.masks import make_identity

F32 = mybir.dt.float32
F32R = mybir.dt.float32r
BF16 = mybir.dt.bfloat16
AF = mybir.ActivationFunctionType
OP = mybir.AluOpType

B, C, T = 4, 512, 4096
NH = 4              # heads per batch
HCH = 128           # channels per head (q/k/v each)
KQ, KP = 32, 5      # conv kernel sizes
PADL_Q, PADR_Q = 15, 16
PADL_P, PADR_P = 2, 2
NG = 32             # groupnorm groups
EPS = 1e-5
N_CORES = 8
CO = 4              # 512 channels = 4 x 128 partitions
TT_ = 8             # t tiles of 512
TN = 512
SCALE2 = float(HCH) ** -0.5   # folded q/k scale applied inside exp


def _round_f32r(a: np.ndarray) -> np.ndarray:
    """Round f32 to the float32r grid: RNE keeping 11 mantissa bits."""
    b = np.ascontiguousarray(a, dtype=np.float32).view(np.uint32).astype(np.uint64)
    shift = 12
    add = (1 << (shift - 1)) - 1 + ((b >> shift) & 1)
    out = (((b + add) >> shift) << shift).astype(np.uint32)
    return out.view(np.float32)


def _split_excess_waits(nc, max_waits=1):
    """Walrus allows one sync wait per instruction; Tile's kernel-tail drain
    can carry several. Move excess ge-imm waits onto preceding NOPs."""
    for f in nc.m.functions:
        for bb in f.blocks:
            insts = bb.instructions
            new_insts = []
            changed = False
            for inst in insts:
                si = inst.sync_info
                if si is not None and len(si.on_wait) > max_waits:
                    waits = list(si.on_wait)
                    movable = [w for w in waits if w.wait_mode == "sem-ge-imm"]
                    sticky = [w for w in waits if w.wait_mode != "sem-ge-imm"]
                    n_keep = max(0, max_waits - len(sticky))
                    keep = movable[:n_keep]
                    rest = movable[n_keep:]
                    for ci in range(0, len(rest), max_waits):
                        nop = mybir.InstNoOp(
                            name=f"{inst.name}-ws{ci}",
                            sync_info=mybir.SyncInfo(
                                on_wait=rest[ci:ci + max_waits], on_update=[]),
                            bass_nofuse=True,
                            engine=inst.engine,
                        )
                        new_insts.append(nop)
                        changed = True
                    si.on_wait = sticky + keep
                    inst.sync_info = si
                new_insts.append(inst)
            if changed:
                bb.instructions = new_insts


def _build_program():
    nc = bass.Bass("TRN2", target_bir_lowering=False, debug=False,
                   num_devices=N_CORES)

    xb_ap = nc.dram_tensor("xb", [C, T], F32, kind="ExternalInput").ap()
    wq_ap = nc.dram_tensor("wq", [6, KQ, CO, 128, 128], F32R, kind="ExternalInput").ap()
    bq_ap = nc.dram_tensor("bq", [128, 6], F32, kind="ExternalInput").ap()
    gam_ap = nc.dram_tensor("gam", [128, CO], F32, kind="ExternalInput").ap()
    bet_ap = nc.dram_tensor("bet", [128, CO], F32, kind="ExternalInput").ap()
    mg_ap = nc.dram_tensor("mg", [128, 8], F32, kind="ExternalInput").ap()
    m2g_ap = nc.dram_tensor("m2g", [8, 128], F32, kind="ExternalInput").ap()
    wp_ap = nc.dram_tensor("wp", [2, KP, CO, 128, 128], F32R, kind="ExternalInput").ap()
    bp_ap = nc.dram_tensor("bp", [128, 2], F32, kind="ExternalInput").ap()
    xr_ap = nc.dram_tensor("xr", [256, T], F32, kind="ExternalInput").ap()
    out_ap = nc.dram_tensor("out", [256, T], F32, kind="ExternalOutput").ap()

    xb_v = xb_ap.rearrange("(co p) t -> p co t", p=128)
    xr_v = xr_ap.rearrange("(ot p) t -> p ot t", p=128)
    out_v = out_ap.rearrange("(ot p) t -> p ot t", p=128)

    GW = PADL_Q + T + PADR_Q          # 4127 padded gn width
    HW = PADL_P + T + PADR_P          # 4100 padded h width

    with tile.TileContext(nc) as tc:
        with tc.tile_pool(name="pconst", bufs=1) as pconst, \
             tc.tile_pool(name="pgn", bufs=1) as pgn, \
             tc.tile_pool(name="pdram", bufs=1, space="DRAM") as pdram:

            gn_sb = pgn.tile([128, CO, GW], F32R, name="gn_sb")
            h_own = pdram.tile([256, T], F32R, name="h_own")
            h_pair = pdram.tile([512, T], F32R, name="h_pair")

            gam_sb = pconst.tile([128, CO], F32, name="gam_sb")
            bet_sb = pconst.tile([128, CO], F32, name="bet_sb")
            bq_sb = pconst.tile([128, 6], F32, name="bq_sb")
            mg_sb = pconst.tile([128, 8], F32, name="mg_sb")
            m2g_sb = pconst.tile([8, 128], F32, name="m2g_sb")
            ident = pconst.tile([128, 128], BF16, name="ident")
            ones_b = pconst.tile([128, 1], BF16, name="ones_b")
            ones_r = pconst.tile([1, 128], F32R, name="ones_r")
            onesf = pconst.tile([1, 128], F32, name="onesf")
            nc.sync.dma_start(gam_sb[:], gam_ap[:])
            nc.sync.dma_start(bet_sb[:], bet_ap[:])
            nc.sync.dma_start(bq_sb[:], bq_ap[:])
            nc.sync.dma_start(mg_sb[:], mg_ap[:])
            nc.sync.dma_start(m2g_sb[:], m2g_ap[:])
            make_identity(nc, ident[:])
            nc.vector.memset(ones_b[:], 1.0)
            nc.vector.memset(onesf[:], 1.0)
            nc.vector.tensor_copy(ones_r[:], onesf[:])

            # ---------------- GroupNorm ----------------
            with tc.tile_pool(name="px", bufs=1) as px, \
                 tc.tile_pool(name="pgs", bufs=1) as pgs, \
                 tc.tile_pool(name="gps", bufs=1, space="PSUM") as gps:
                x_sb = px.tile([128, CO, T], F32, name="x_sb")
                nc.sync.dma_start(x_sb[:], xb_v[:])
                s_tile = pgs.tile([128, 8], F32, name="s_tile")
                for co in range(CO):
                    nc.vector.reduce_sum(
                        out=s_tile[:, co:co + 1], in_=x_sb[:, co, :],
                        axis=mybir.AxisListType.X)
                for co in range(CO):
                    sq = px.tile([128, T], F32, name="sq", tag="sq", bufs=2)
                    nc.scalar.activation(sq[:], x_sb[:, co, :], AF.Square,
                                         accum_out=s_tile[:, 4 + co:5 + co])

                g_ps = gps.tile([8, 8], F32, name="g_ps", tag="gps8")
                nc.tensor.matmul(g_ps[:], mg_sb[:], s_tile[:], start=True, stop=True)
                # group moments -> per-group mean/rstd on 8 partitions
                mr = pgs.tile([8, 8], F32, name="mr")          # [:,0:4]=mean, [:,4:8]=rstd
                var = pgs.tile([8, 4], F32, name="var")
                tmp = pgs.tile([8, 4], F32, name="tmpg")
                inv_n = 1.0 / (16.0 * T)
                nc.vector.tensor_scalar_mul(mr[:, 0:4], g_ps[:, 0:4], inv_n)
                nc.vector.tensor_scalar_mul(var[:], g_ps[:, 4:8], inv_n)
                nc.vector.tensor_mul(out=tmp[:], in0=mr[:, 0:4], in1=mr[:, 0:4])
                nc.vector.tensor_sub(out=var[:], in0=var[:], in1=tmp[:])
                nc.vector.tensor_scalar_add(var[:], var[:], EPS)
                rec = pgs.tile([8, 4], F32, name="rec")
                nc.vector.reciprocal(out=rec[:], in_=var[:])
                nc.scalar.activation(mr[:, 4:8], rec[:], AF.Sqrt)
                # one Newton step: r <- r * (1.5 - 0.5 * var * r^2)
                nc.vector.tensor_mul(out=tmp[:], in0=mr[:, 4:8], in1=mr[:, 4:8])
                nc.vector.tensor_mul(out=tmp[:], in0=tmp[:], in1=var[:])
                nc.vector.tensor_scalar(tmp[:], tmp[:], -0.5, 1.5, OP.mult, OP.add)
                nc.vector.tensor_mul(out=mr[:, 4:8], in0=mr[:, 4:8], in1=tmp[:])

                pc_ps = gps.tile([128, 8], F32, name="pc_ps", tag="gpc")
                nc.tensor.matmul(pc_ps[:], m2g_sb[:], mr[:], start=True, stop=True)
                pc = pgs.tile([128, 8], F32, name="pc")
                nc.vector.tensor_copy(pc[:], pc_ps[:])
                a_sb = pgs.tile([128, CO], F32, name="a_sb")
                b2_sb = pgs.tile([128, CO], F32, name="b2_sb")
                nc.vector.tensor_mul(out=a_sb[:], in0=pc[:, 4:8], in1=gam_sb[:])
                nc.vector.tensor_mul(out=b2_sb[:], in0=pc[:, 0:4], in1=a_sb[:])
                nc.vector.tensor_sub(out=b2_sb[:], in0=bet_sb[:], in1=b2_sb[:])

                # zero halos (via DVE so the f32r-rounded-producer check passes)
                zh = pgs.tile([128, CO, 16], F32, name="zh")
                nc.vector.memset(zh[:], 0.0)
                nc.vector.tensor_copy(gn_sb[:, :, 0:PADL_Q], zh[:, :, 0:PADL_Q])
                nc.vector.tensor_copy(gn_sb[:, :, PADL_Q + T:GW], zh[:, :, 0:PADR_Q])
                for co in range(CO):
                    nc.vector.tensor_scalar(
                        gn_sb[:, co, PADL_Q:PADL_Q + T], x_sb[:, co, :],
                        a_sb[:, co:co + 1], b2_sb[:, co:co + 1], OP.mult, OP.add)

            # ---------------- QKV conv + attention ----------------
            with tc.tile_pool(name="pwq", bufs=4) as pwq, \
                 tc.tile_pool(name="pqkv", bufs=1) as pqkv, \
                 tc.tile_pool(name="pat", bufs=1) as pat, \
                 tc.tile_pool(name="cps", bufs=4, space="PSUM") as cps, \
                 tc.tile_pool(name="aps", bufs=1, space="PSUM") as aps:

                h_own_v = h_own[:].rearrange("(hl p) t -> p hl t", p=128)

                for hl in range(2):
                    qkv_tiles = {}
                    for lk, kind in enumerate("qkv"):
                        ot = hl * 3 + lk
                        dt_ = BF16 if kind == "v" else F32R
                        dst = pqkv.tile([128, T], dt_, name=f"{kind}{hl}_sb")
                        qkv_tiles[kind] = dst
                        for th in range(2):
                            pc_tiles = [
                                cps.tile([128, TN], F32, name=f"cv{ot}_{th}_{t4}",
                                         tag="cv", bufs=4)
                                for t4 in range(4)
                            ]
                            for j in range(KQ):
                                w_t = pwq.tile([128, CO, 128], F32R,
                                               name=f"w{ot}_{th}_{j}", tag="wq", bufs=4)
                                nc.sync.dma_start(
                                    w_t[:], wq_ap[ot, j].rearrange("co c o -> c co o"))
                                for co in range(CO):
                                    for t4 in range(4):
                                        t0 = th * 2048 + t4 * TN
                                        nc.tensor.matmul(
                                            pc_tiles[t4][:],
                                            w_t[:, co, :],
                                            gn_sb[:, co, t0 + j:t0 + j + TN],
                                            start=(j == 0 and co == 0),
                                            stop=(j == KQ - 1 and co == CO - 1))
                            for t4 in range(4):
                                t0 = th * 2048 + t4 * TN
                                nc.vector.tensor_scalar_add(
                                    dst[:, t0:t0 + TN], pc_tiles[t4][:],
                                    bq_sb[:, ot:ot + 1])

                    # attention for this head
                    q_sb, k_sb, v_sb = qkv_tiles["q"], qkv_tiles["k"], qkv_tiles["v"]
                    vT = pat.tile([128, 32, 128], BF16, name=f"vT{hl}", tag="vT", bufs=2)
                    for sb in range(32):
                        tp = cps.tile([128, 128], BF16, name=f"tp{hl}_{sb}",
                                      tag="cv", bufs=4)
                        nc.tensor.transpose(tp[:], v_sb[:, sb * 128:(sb + 1) * 128],
                                            ident[:])
                        nc.vector.tensor_copy(vT[:, sb, :], tp[:])

                    for t8 in range(TT_):
                        t0 = t8 * TN
                        h_ps = aps.tile([128, TN], F32, name=f"h_ps{hl}_{t8}",
                                        tag="hps", bufs=1)
                        d_ps = aps.tile([128, TN], F32, name=f"d_ps{hl}_{t8}",
                                        tag="dps", bufs=1)
                        for sb in range(32):
                            pt_ps = aps.tile([128, TN], F32, name=f"pt{hl}_{t8}_{sb}",
                                             tag="ptps", bufs=2)
                            nc.tensor.matmul(
                                pt_ps[:], k_sb[:, sb * 128:(sb + 1) * 128],
                                q_sb[:, t0:t0 + TN], start=True, stop=True)
                            pt_sb = pat.tile([128, TN], BF16, name=f"pts{hl}_{t8}_{sb}",
                                             tag="pts", bufs=3)
                            nc.scalar.activation(pt_sb[:], pt_ps[:], AF.Exp,
                                                 scale=SCALE2)
                            nc.tensor.matmul(h_ps[:], vT[:, sb, :], pt_sb[:],
                                             start=(sb == 0), stop=(sb == 31))
                            nc.tensor.matmul(d_ps[0:1, :], ones_b[:], pt_sb[:],
                                             start=(sb == 0), stop=(sb == 31))
                        rd = pat.tile([1, TN], F32R, name=f"rd{hl}_{t8}", tag="rd",
                                      bufs=2)
                        with nc.allow_low_precision(
                                reason="f32r rdenom: 1e-4 rel is plenty"):
                            nc.vector.reciprocal(out=rd[:], in_=d_ps[0:1, :])
                        nc.tensor.matmul(d_ps[:], ones_r[:], rd[:], start=True,
                                         stop=True)
                        r_sb = pat.tile([128, TN], F32, name=f"rs{hl}_{t8}", tag="rs",
                                        bufs=2)
                        nc.vector.tensor_copy(r_sb[:], d_ps[:])
                        hn = pat.tile([128, TN], F32R, name=f"hn{hl}_{t8}", tag="hn",
                                      bufs=2)
                        nc.vector.tensor_mul(out=hn[:], in0=h_ps[:], in1=r_sb[:])
                        nc.sync.dma_start(h_own_v[:, hl, t0:t0 + TN], hn[:])

                # pair exchange of attention outputs
                nc.gpsimd.collective_compute(
                    "AllGather", OP.bypass,
                    replica_groups=[[0, 1], [2, 3], [4, 5], [6, 7]],
                    ins=[h_own[:].opt()], outs=[h_pair[:].opt()])

            # ---------------- proj conv + residual ----------------
            with tc.tile_pool(name="pproj", bufs=1) as ppj, \
                 tc.tile_pool(name="pps", bufs=2, space="PSUM") as pps:
                h_sb = ppj.tile([128, CO, HW], F32R, name="h_sb")
                zh2 = ppj.tile([128, CO, 2], F32, name="zh2")
                nc.vector.memset(zh2[:], 0.0)
                nc.vector.tensor_copy(h_sb[:, :, 0:PADL_P], zh2[:])
                nc.vector.tensor_copy(h_sb[:, :, PADL_P + T:HW], zh2[:])
                nc.sync.dma_start(
                    h_sb[:, :, PADL_P:PADL_P + T],
                    h_pair[:].rearrange("(co p) t -> p co t", p=128))
                pw_sb = ppj.tile([128, 2, KP, CO, 128], F32R, name="pw_sb")
                nc.sync.dma_start(
                    pw_sb[:], wp_ap[:].rearrange("ot j co c o -> c ot j co o"))
                bp_sb = ppj.tile([128, 2], F32, name="bp_sb")
                nc.sync.dma_start(bp_sb[:], bp_ap[:])
                xf_sb = ppj.tile([128, 2, T], F32, name="xf_sb")
                nc.sync.dma_start(xf_sb[:], xr_v[:])

                for ot in range(2):
                    for t8 in range(TT_):
                        t0 = t8 * TN
                        pp = pps.tile([128, TN], F32, name=f"pp{ot}_{t8}",
                                      tag="pp", bufs=2)
                        for j in range(KP):
                            for co in range(CO):
                                nc.tensor.matmul(
                                    pp[:], pw_sb[:, ot, j, co, :],
                                    h_sb[:, co, t0 + j:t0 + j + TN],
                                    start=(j == 0 and co == 0),
                                    stop=(j == KP - 1 and co == CO - 1))
                        o1 = ppj.tile([128, TN], F32, name=f"o1_{ot}_{t8}",
                                      tag="o1", bufs=3)
                        nc.vector.tensor_scalar_add(o1[:], pp[:], bp_sb[:, ot:ot + 1])
                        nc.vector.tensor_add(out=o1[:], in0=o1[:],
                                             in1=xf_sb[:, ot, t0:t0 + TN])
                        nc.sync.dma_start(out_v[:, ot, t0:t0 + TN], o1[:])

    _split_excess_waits(nc, max_waits=1)
    return nc


@functools.lru_cache(maxsize=1)
def _get_program():
    return _build_program()


def _prepare_inputs(x, gn_gamma, gn_beta, qkv_w, qkv_b, proj_w, proj_b):
    x = np.ascontiguousarray(x, dtype=np.float32).reshape(B, C, T)
    qkv_w_r = _round_f32r(qkv_w)                      # [1536, 512, 32]
    proj_w_r = _round_f32r(proj_w)                    # [512, 512, 5]

    gam_pc = np.ascontiguousarray(gn_gamma.reshape(CO, 128).T, dtype=np.float32)
    bet_pc = np.ascontiguousarray(gn_beta.reshape(CO, 128).T, dtype=np.float32)
    mg = np.zeros((128, 8), dtype=np.float32)
    for p in range(128):
        mg[p, p // 16] = 1.0
    m2g = np.ascontiguousarray(mg.T)

    in_maps = []
    for c in range(N_CORES):
        b = c // 2
        h0 = 2 * (c % 2)
        ohalf = c % 2
        # [768, 512, 32] -> [6 ot, 32 j, 4 co, 128 c, 128 o]
        wq = qkv_w_r[384 * h0:384 * h0 + 768]
        wq = np.ascontiguousarray(
            wq.reshape(6, 128, CO, 128, KQ).transpose(0, 4, 2, 3, 1))
        bq = np.ascontiguousarray(
            qkv_b[384 * h0:384 * h0 + 768].reshape(6, 128).T, dtype=np.float32)
        wp = proj_w_r[256 * ohalf:256 * ohalf + 256]   # [256, 512, 5]
        wp = np.ascontiguousarray(
            wp.reshape(2, 128, CO, 128, KP).transpose(0, 4, 2, 3, 1))
        bp = np.ascontiguousarray(
            proj_b[256 * ohalf:256 * ohalf + 256].reshape(2, 128).T,
            dtype=np.float32)
        xr = np.ascontiguousarray(x[b, 256 * ohalf:256 * ohalf + 256, :])
        in_maps.append({
            "xb": x[b], "wq": wq, "bq": bq,
            "gam": gam_pc, "bet": bet_pc, "mg": mg, "m2g": m2g,
            "wp": wp, "bp": bp, "xr": xr,
        })
    return in_maps


def _run(in_maps, trace=False, **kw):
    nc = _get_program()
    return bass_utils.run_bass_kernel_spmd(
        nc, in_maps, core_ids=list(range(N_CORES)), trace=trace, **kw)


def kernel(x, gn_gamma, gn_beta, qkv_w, qkv_b, proj_w, proj_b):
    in_maps = _prepare_inputs(np.asarray(x), np.asarray(gn_gamma),
                              np.asarray(gn_beta), np.asarray(qkv_w),
                              np.asarray(qkv_b), np.asarray(proj_w),
                              np.asarray(proj_b))
    res = _run(in_maps)
    out = np.empty((B, C, T), dtype=np.float32)
    for c in range(N_CORES):
        b, ohalf = c // 2, c % 2
        out[b, 256 * ohalf:256 * ohalf + 256, :] = res.results[c]["out"]
    return out.reshape(B, C, 64, 64)


# revision 7
# speedup vs baseline: 1.2567x; 1.2567x over previous
"""AttentionBlock (GroupNorm -> conv1d k=32 QKV -> 16-head attention over t=4096
-> conv1d k=5 proj -> residual) on 8 Trainium2 NeuronCores.

Sharding: 16 (batch, head) attention tasks over 8 cores -> 2 heads per core
(core c: batch c//2, heads 2*(c%2), 2*(c%2)+1). Each core computes GroupNorm
for its batch, the QKV conv restricted to its heads' 768 output channels
(expressed as 128 accumulated matmuls per PSUM tile), and attention for its 2
heads. The proj conv needs all 512 h-channels, so core pairs AllGather h, then
each core computes a 256-output-channel half of the proj conv + residual.

Matmuls run in float32r (full PE rate at N=512, ~1e-4 relative error);
attention probabilities/AV run in bf16. Host pre-rounds f32r weights
(RNE to 11 mantissa bits, verified bit-exact against the device cast).
"""

import functools
import os
import numpy as np

import concourse.bass as bass
import concourse.mybir as mybir
import concourse.tile as tile
from concourse import bass_utils
from concourse.masks import make_identity

F32 = mybir.dt.float32
F32R = mybir.dt.float32r
F16 = mybir.dt.float16
BF16 = mybir.dt.bfloat16
AF = mybir.ActivationFunctionType
OP = mybir.AluOpType

B, C, T = 4, 512, 4096
NH = 4              # heads per batch
HCH = 128           # channels per head (q/k/v each)
KQ, KP = 32, 5      # conv kernel sizes
PADL_Q, PADR_Q = 15, 16
PADL_P, PADR_P = 2, 2
NG = 32             # groupnorm groups
EPS = 1e-5
N_CORES = 8
CO = 4              # 512 channels = 4 x 128 partitions
TT_ = 8             # t tiles of 512
TN = 512
SCALE2 = float(HCH) ** -0.5   # folded q/k scale applied inside exp


def _round_f32r(a: np.ndarray) -> np.ndarray:
    """Round f32 to the float32r grid: RNE keeping 11 mantissa bits."""
    b = np.ascontiguousarray(a, dtype=np.float32).view(np.uint32).astype(np.uint64)
    shift = 12
    add = (1 << (shift - 1)) - 1 + ((b >> shift) & 1)
    out = (((b + add) >> shift) << shift).astype(np.uint32)
    return out.view(np.float32)


def _split_excess_waits(nc, max_waits=1):
    """Walrus allows one sync wait per instruction; Tile's kernel-tail drain
    can carry several. Move excess ge-imm waits onto preceding NOPs."""
    for f in nc.m.functions:
        for bb in f.blocks:
            insts = bb.instructions
            new_insts = []
            changed = False
            for inst in insts:
                si = inst.sync_info
                if si is not None and len(si.on_wait) > max_waits:
                    waits = list(si.on_wait)
                    movable = [w for w in waits if w.wait_mode == "sem-ge-imm"]
                    sticky = [w for w in waits if w.wait_mode != "sem-ge-imm"]
                    n_keep = max(0, max_waits - len(sticky))
                    keep = movable[:n_keep]
                    rest = movable[n_keep:]
                    for ci in range(0, len(rest), max_waits):
                        nop = mybir.InstNoOp(
                            name=f"{inst.name}-ws{ci}",
                            sync_info=mybir.SyncInfo(
                                on_wait=rest[ci:ci + max_waits], on_update=[]),
                            bass_nofuse=True,
                            engine=inst.engine,
                        )
                        new_insts.append(nop)
                        changed = True
                    si.on_wait = sticky + keep
                    inst.sync_info = si
                new_insts.append(inst)
            if changed:
                bb.instructions = new_insts


def _build_program():
    nc = bass.Bass("TRN2", target_bir_lowering=False, debug=False,
                   num_devices=N_CORES)

    xb_ap = nc.dram_tensor("xb", [C, T], F32, kind="ExternalInput").ap()
    wq_ap = nc.dram_tensor("wq", [6, KQ, CO, 128, 128], F16, kind="ExternalInput").ap()
    bq_ap = nc.dram_tensor("bq", [128, 6], F32, kind="ExternalInput").ap()
    gam_ap = nc.dram_tensor("gam", [128, CO], F32, kind="ExternalInput").ap()
    bet_ap = nc.dram_tensor("bet", [128, CO], F32, kind="ExternalInput").ap()
    mg_ap = nc.dram_tensor("mg", [128, 8], F32, kind="ExternalInput").ap()
    m2g_ap = nc.dram_tensor("m2g", [8, 128], F32, kind="ExternalInput").ap()
    wp_ap = nc.dram_tensor("wp", [2, KP, CO, 128, 128], F16, kind="ExternalInput").ap()
    bp_ap = nc.dram_tensor("bp", [128, 2], F32, kind="ExternalInput").ap()
    xr_ap = nc.dram_tensor("xr", [256, T], F32, kind="ExternalInput").ap()
    out_ap = nc.dram_tensor("out", [256, T], F32, kind="ExternalOutput").ap()
    debug = os.environ.get("KDEBUG", "0") == "1"
    if debug:
        gn_dump = nc.dram_tensor("gn_dump", [128, CO, PADL_Q + T + PADR_Q], F16, kind="ExternalOutput").ap()
        q_dump = nc.dram_tensor("q_dump", [128, T], F16, kind="ExternalOutput").ap()
        k_dump = nc.dram_tensor("k_dump", [128, T], F16, kind="ExternalOutput").ap()
        v_dump = nc.dram_tensor("v_dump", [128, T], BF16, kind="ExternalOutput").ap()
        h_dump = nc.dram_tensor("h_dump", [256, T], F32, kind="ExternalOutput").ap()

    xb_v = xb_ap.rearrange("(co p) t -> p co t", p=128)
    xr_v = xr_ap.rearrange("(ot p) t -> p ot t", p=128)
    out_v = out_ap.rearrange("(ot p) t -> p ot t", p=128)

    GW = PADL_Q + T + PADR_Q          # 4127 padded gn width
    HW = PADL_P + T + PADR_P          # 4100 padded h width

    with tile.TileContext(nc) as tc:
        with tc.tile_pool(name="pconst", bufs=1) as pconst, \
             tc.tile_pool(name="pgn", bufs=1) as pgn, \
             tc.tile_pool(name="pdram", bufs=1, space="DRAM") as pdram:

            gn_sb = pgn.tile([128, CO, GW], F16, name="gn_sb")
            h_own = pdram.tile([256, T], F32, name="h_own")
            h_pair = pdram.tile([512, T], F32, name="h_pair")

            gam_sb = pconst.tile([128, CO], F32, name="gam_sb")
            bet_sb = pconst.tile([128, CO], F32, name="bet_sb")
            bq_sb = pconst.tile([128, 6], F32, name="bq_sb")
            mg_sb = pconst.tile([128, 8], F32, name="mg_sb")
            m2g_sb = pconst.tile([8, 128], F32, name="m2g_sb")
            ident = pconst.tile([128, 128], BF16, name="ident")
            ones_b = pconst.tile([128, 1], BF16, name="ones_b")
            ones_r = pconst.tile([1, 128], F16, name="ones_r")
            onesf = pconst.tile([1, 128], F32, name="onesf")
            nc.sync.dma_start(gam_sb[:], gam_ap[:])
            nc.sync.dma_start(bet_sb[:], bet_ap[:])
            nc.sync.dma_start(bq_sb[:], bq_ap[:])
            nc.sync.dma_start(mg_sb[:], mg_ap[:])
            nc.sync.dma_start(m2g_sb[:], m2g_ap[:])
            make_identity(nc, ident[:])
            nc.vector.memset(ones_b[:], 1.0)
            nc.vector.memset(onesf[:], 1.0)
            nc.vector.memset(ones_r[:], 1.0)

            # ---------------- GroupNorm ----------------
            with tc.tile_pool(name="px", bufs=1) as px, \
                 tc.tile_pool(name="pgs", bufs=1) as pgs, \
                 tc.tile_pool(name="gps", bufs=1, space="PSUM") as gps:
                x_sb = px.tile([128, CO, T], F32, name="x_sb")
                nc.sync.dma_start(x_sb[:], xb_v[:])
                s_tile = pgs.tile([128, 8], F32, name="s_tile")
                for co in range(CO):
                    nc.vector.reduce_sum(
                        out=s_tile[:, co:co + 1], in_=x_sb[:, co, :],
                        axis=mybir.AxisListType.X)
                for co in range(CO):
                    sq = px.tile([128, T], F32, name="sq", tag="sq", bufs=2)
                    nc.scalar.activation(sq[:], x_sb[:, co, :], AF.Square,
                                         accum_out=s_tile[:, 4 + co:5 + co])

                g_ps = gps.tile([8, 8], F32, name="g_ps", tag="gps8")
                nc.tensor.matmul(g_ps[:], mg_sb[:], s_tile[:], start=True, stop=True)
                # group moments -> per-group mean/rstd on 8 partitions
                mr = pgs.tile([8, 8], F32, name="mr")          # [:,0:4]=mean, [:,4:8]=rstd
                var = pgs.tile([8, 4], F32, name="var")
                tmp = pgs.tile([8, 4], F32, name="tmpg")
                inv_n = 1.0 / (16.0 * T)
                nc.vector.tensor_scalar_mul(mr[:, 0:4], g_ps[:, 0:4], inv_n)
                nc.vector.tensor_scalar_mul(var[:], g_ps[:, 4:8], inv_n)
                nc.vector.tensor_mul(out=tmp[:], in0=mr[:, 0:4], in1=mr[:, 0:4])
                nc.vector.tensor_sub(out=var[:], in0=var[:], in1=tmp[:])
                nc.vector.tensor_scalar_add(var[:], var[:], EPS)
                rec = pgs.tile([8, 4], F32, name="rec")
                nc.vector.reciprocal(out=rec[:], in_=var[:])
                nc.scalar.activation(mr[:, 4:8], rec[:], AF.Sqrt)
                # one Newton step: r <- r * (1.5 - 0.5 * var * r^2)
                nc.vector.tensor_mul(out=tmp[:], in0=mr[:, 4:8], in1=mr[:, 4:8])
                nc.vector.tensor_mul(out=tmp[:], in0=tmp[:], in1=var[:])
                nc.vector.tensor_scalar(tmp[:], tmp[:], -0.5, 1.5, OP.mult, OP.add)
                nc.vector.tensor_mul(out=mr[:, 4:8], in0=mr[:, 4:8], in1=tmp[:])

                pc_ps = gps.tile([128, 8], F32, name="pc_ps", tag="gpc")
                nc.tensor.matmul(pc_ps[:], m2g_sb[:], mr[:], start=True, stop=True)
                pc = pgs.tile([128, 8], F32, name="pc")
                nc.vector.tensor_copy(pc[:], pc_ps[:])
                a_sb = pgs.tile([128, CO], F32, name="a_sb")
                b2_sb = pgs.tile([128, CO], F32, name="b2_sb")
                nc.vector.tensor_mul(out=a_sb[:], in0=pc[:, 4:8], in1=gam_sb[:])
                nc.vector.tensor_mul(out=b2_sb[:], in0=pc[:, 0:4], in1=a_sb[:])
                nc.vector.tensor_sub(out=b2_sb[:], in0=bet_sb[:], in1=b2_sb[:])

                # zero halos (via DVE so the f32r-rounded-producer check passes)
                zh = pgs.tile([128, CO, 16], F32, name="zh")
                nc.vector.memset(zh[:], 0.0)
                nc.vector.tensor_copy(gn_sb[:, :, 0:PADL_Q], zh[:, :, 0:PADL_Q])
                nc.vector.tensor_copy(gn_sb[:, :, PADL_Q + T:GW], zh[:, :, 0:PADR_Q])
                for co in range(CO):
                    nc.vector.tensor_scalar(
                        gn_sb[:, co, PADL_Q:PADL_Q + T], x_sb[:, co, :],
                        a_sb[:, co:co + 1], b2_sb[:, co:co + 1], OP.mult, OP.add)

            # ---------------- QKV conv + attention ----------------
            with tc.tile_pool(name="pwq", bufs=4) as pwq, \
                 tc.tile_pool(name="pqkv", bufs=1) as pqkv, \
                 tc.tile_pool(name="pat", bufs=1) as pat, \
                 tc.tile_pool(name="cps", bufs=4, space="PSUM") as cps, \
                 tc.tile_pool(name="aps", bufs=1, space="PSUM") as aps:

                h_own_v = h_own[:].rearrange("(hl p) t -> p hl t", p=128)

                for hl in range(2):
                    qkv_tiles = {}
                    for lk, kind in enumerate("qkv"):
                        ot = hl * 3 + lk
                        dt_ = BF16 if kind == "v" else F16
                        dst = pqkv.tile([128, T], dt_, name=f"{kind}{hl}_sb")
                        qkv_tiles[kind] = dst
                        for th in range(2):
                            pc_tiles = [
                                cps.tile([128, TN], F32, name=f"cv{ot}_{th}_{t4}",
                                         tag="cv", bufs=4)
                                for t4 in range(4)
                            ]
                            for j in range(KQ):
                                w_t = pwq.tile([128, CO, 128], F16,
                                               name=f"w{ot}_{th}_{j}", tag="wq", bufs=6)
                                nc.sync.dma_start(
                                    w_t[:], wq_ap[ot, j].rearrange("co c o -> c co o"))
                                for co in range(CO):
                                    for t4 in range(4):
                                        t0 = th * 2048 + t4 * TN
                                        nc.tensor.matmul(
                                            pc_tiles[t4][:],
                                            w_t[:, co, :],
                                            gn_sb[:, co, t0 + j:t0 + j + TN],
                                            start=(j == 0 and co == 0),
                                            stop=(j == KQ - 1 and co == CO - 1))
                            for t4 in range(4):
                                t0 = th * 2048 + t4 * TN
                                nc.vector.tensor_scalar_add(
                                    dst[:, t0:t0 + TN], pc_tiles[t4][:],
                                    bq_sb[:, ot:ot + 1])

                    # attention for this head
                    q_sb, k_sb, v_sb = qkv_tiles["q"], qkv_tiles["k"], qkv_tiles["v"]
                    if debug and hl == 0:
                        nc.sync.dma_start(q_dump[:], q_sb[:])
                        nc.sync.dma_start(k_dump[:], k_sb[:])
                        nc.sync.dma_start(v_dump[:], v_sb[:])
                    vT = pat.tile([128, 32, 128], BF16, name=f"vT{hl}", tag="vT", bufs=2)
                    for sb in range(32):
                        tp = cps.tile([128, 128], BF16, name=f"tp{hl}_{sb}",
                                      tag="cv", bufs=4)
                        nc.tensor.transpose(tp[:], v_sb[:, sb * 128:(sb + 1) * 128],
                                            ident[:])
                        nc.vector.tensor_copy(vT[:, sb, :], tp[:])

                    for t8 in range(TT_):
                        t0 = t8 * TN
                        h_ps = aps.tile([128, TN], F32, name=f"h_ps{hl}_{t8}",
                                        tag="hps", bufs=1)
                        d_ps = aps.tile([128, TN], F32, name=f"d_ps{hl}_{t8}",
                                        tag="dps", bufs=1)
                        for sb in range(32):
                            pt_ps = aps.tile([128, TN], F32, name=f"pt{hl}_{t8}_{sb}",
                                             tag="ptps", bufs=2)
                            nc.tensor.matmul(
                                pt_ps[:], k_sb[:, sb * 128:(sb + 1) * 128],
                                q_sb[:, t0:t0 + TN], start=True, stop=True)
                            pt_sb = pat.tile([128, TN], BF16, name=f"pts{hl}_{t8}_{sb}",
                                             tag="pts", bufs=4)
                            nc.scalar.activation(pt_sb[:], pt_ps[:], AF.Exp,
                                                 scale=SCALE2)
                            nc.tensor.matmul(h_ps[:], vT[:, sb, :], pt_sb[:],
                                             start=(sb == 0), stop=(sb == 31))
                            nc.tensor.matmul(d_ps[0:1, :], ones_b[:], pt_sb[:],
                                             start=(sb == 0), stop=(sb == 31))
                        rd = pat.tile([1, TN], F32, name=f"rd{hl}_{t8}", tag="rd",
                                      bufs=2)
                        nc.vector.reciprocal(out=rd[:], in_=d_ps[0:1, :])
                        nc.tensor.matmul(d_ps[:], onesf[:], rd[:], start=True,
                                         stop=True)
                        r_sb = pat.tile([128, TN], F32, name=f"rs{hl}_{t8}", tag="rs",
                                        bufs=2)
                        nc.vector.tensor_copy(r_sb[:], d_ps[:])
                        hn = pat.tile([128, TN], F32, name=f"hn{hl}_{t8}", tag="hn",
                                      bufs=2)
                        nc.vector.tensor_mul(out=hn[:], in0=h_ps[:], in1=r_sb[:])
                        nc.sync.dma_start(h_own_v[:, hl, t0:t0 + TN], hn[:])

                if debug:
                    nc.sync.dma_start(gn_dump[:], gn_sb[:])
                    nc.sync.dma_start(h_dump[:], h_own[:])
                # pair exchange of attention outputs
                nc.gpsimd.collective_compute(
                    "AllGather", OP.bypass,
                    replica_groups=[[0, 1], [2, 3], [4, 5], [6, 7]],
                    ins=[h_own[:].opt()], outs=[h_pair[:].opt()])

            # ---------------- proj conv + residual ----------------
            with tc.tile_pool(name="pproj", bufs=1) as ppj, \
                 tc.tile_pool(name="pps", bufs=2, space="PSUM") as pps:
                h_sb = ppj.tile([128, CO, HW], F16, name="h_sb")
                zh2 = ppj.tile([128, CO, 2], F32, name="zh2")
                nc.vector.memset(zh2[:], 0.0)
                nc.vector.tensor_copy(h_sb[:, :, 0:PADL_P], zh2[:])
                nc.vector.tensor_copy(h_sb[:, :, PADL_P + T:HW], zh2[:])
                h_f32 = ppj.tile([128, CO, T], F32, name="h_f32")
                nc.sync.dma_start(
                    h_f32[:], h_pair[:].rearrange("(co p) t -> p co t", p=128))
                nc.vector.tensor_copy(h_sb[:, :, PADL_P:PADL_P + T], h_f32[:])
                pw_sb = ppj.tile([128, 2, KP, CO, 128], F16, name="pw_sb")
                nc.sync.dma_start(
                    pw_sb[:], wp_ap[:].rearrange("ot j co c o -> c ot j co o"))
                bp_sb = ppj.tile([128, 2], F32, name="bp_sb")
                nc.sync.dma_start(bp_sb[:], bp_ap[:])
                xf_sb = ppj.tile([128, 2, T], F32, name="xf_sb")
                nc.sync.dma_start(xf_sb[:], xr_v[:])

                for ot in range(2):
                    for t8 in range(TT_):
                        t0 = t8 * TN
                        pp = pps.tile([128, TN], F32, name=f"pp{ot}_{t8}",
                                      tag="pp", bufs=2)
                        for j in range(KP):
                            for co in range(CO):
                                nc.tensor.matmul(
                                    pp[:], pw_sb[:, ot, j, co, :],
                                    h_sb[:, co, t0 + j:t0 + j + TN],
                                    start=(j == 0 and co == 0),
                                    stop=(j == KP - 1 and co == CO - 1))
                        o1 = ppj.tile([128, TN], F32, name=f"o1_{ot}_{t8}",
                                      tag="o1", bufs=3)
                        nc.vector.tensor_scalar_add(o1[:], pp[:], bp_sb[:, ot:ot + 1])
                        nc.vector.tensor_add(out=o1[:], in0=o1[:],
                                             in1=xf_sb[:, ot, t0:t0 + TN])
                        nc.sync.dma_start(out_v[:, ot, t0:t0 + TN], o1[:])

    _split_excess_waits(nc, max_waits=1)
    return nc


@functools.lru_cache(maxsize=1)
def _get_program():
    return _build_program()


def _prepare_inputs(x, gn_gamma, gn_beta, qkv_w, qkv_b, proj_w, proj_b):
    x = np.ascontiguousarray(x, dtype=np.float32).reshape(B, C, T)
    qkv_w_r = np.asarray(qkv_w, dtype=np.float16)     # [1536, 512, 32]
    proj_w_r = np.asarray(proj_w, dtype=np.float16)   # [512, 512, 5]

    gam_pc = np.ascontiguousarray(gn_gamma.reshape(CO, 128).T, dtype=np.float32)
    bet_pc = np.ascontiguousarray(gn_beta.reshape(CO, 128).T, dtype=np.float32)
    mg = np.zeros((128, 8), dtype=np.float32)
    for p in range(128):
        mg[p, p // 16] = 1.0
    m2g = np.ascontiguousarray(mg.T)

    in_maps = []
    for c in range(N_CORES):
        b = c // 2
        h0 = 2 * (c % 2)
        ohalf = c % 2
        # [768, 512, 32] -> [6 ot, 32 j, 4 co, 128 c, 128 o]
        wq = qkv_w_r[384 * h0:384 * h0 + 768]
        wq = np.ascontiguousarray(
            wq.reshape(6, 128, CO, 128, KQ).transpose(0, 4, 2, 3, 1))
        bq = np.ascontiguousarray(
            qkv_b[384 * h0:384 * h0 + 768].reshape(6, 128).T, dtype=np.float32)
        wp = proj_w_r[256 * ohalf:256 * ohalf + 256]   # [256, 512, 5]
        wp = np.ascontiguousarray(
            wp.reshape(2, 128, CO, 128, KP).transpose(0, 4, 2, 3, 1))
        bp = np.ascontiguousarray(
            proj_b[256 * ohalf:256 * ohalf + 256].reshape(2, 128).T,
            dtype=np.float32)
        xr = np.ascontiguousarray(x[b, 256 * ohalf:256 * ohalf + 256, :])
        in_maps.append({
            "xb": x[b], "wq": wq, "bq": bq,
            "gam": gam_pc, "bet": bet_pc, "mg": mg, "m2g": m2g,
            "wp": wp, "bp": bp, "xr": xr,
        })
    return in_maps


def _run(in_maps, trace=False, **kw):
    nc = _get_program()
    return bass_utils.run_bass_kernel_spmd(
        nc, in_maps, core_ids=list(range(N_CORES)), trace=trace, **kw)


def kernel(x, gn_gamma, gn_beta, qkv_w, qkv_b, proj_w, proj_b):
    in_maps = _prepare_inputs(np.asarray(x), np.asarray(gn_gamma),
                              np.asarray(gn_beta), np.asarray(qkv_w),
                              np.asarray(qkv_b), np.asarray(proj_w),
                              np.asarray(proj_b))
    res = _run(in_maps)
    out = np.empty((B, C, T), dtype=np.float32)
    for c in range(N_CORES):
        b, ohalf = c // 2, c % 2
        out[b, 256 * ohalf:256 * ohalf + 256, :] = res.results[c]["out"]
    return out.reshape(B, C, 64, 64)
